# revision 29
# baseline (speedup 1.0000x reference)
"""Trainium2 Bass kernel for nn_ExponentialTrajectoryFit.

Reference computation (dim = 8192):
    d0 = x1-x0; d1 = x2-x1; d2 = x3-x2
    M = [d1 d2] @ pinv([d0 d1])        # dense rank-2 [dim, dim]
    B = I - M;  z = x1 - M x0
    out = x0 - solve(B, z)

Closed form used here (Woodbury + algebraic collapse):
    With D1 = [d0 d1], D2 = [d1 d2], G = D1^T D1, T = D1^T D2, K = G - T,
    the Woodbury rewrite of solve(I - D2 pinv(D1), z) collapses to
        out = a*d1 + b*d2 - d0,   [a, b] = -K^-1 [g00, g01]
    (the correction terms telescope: ab = K^-1 D1^T (x0 - x1)
     = -K^-1 D1^T d0 = -K^-1 [g00, g01]).
    Only FIVE dot products are needed:
        g00=d0.d0  g01=d0.d1  t01=d0.d2  g11=d1.d1  t11=d1.d2
        K = [[g00-g01, g01-t01], [g01-g11, g11-t11]]
    float32 end-to-end agrees with the dense fp32 reference to ~3e-7.

Sign convention on device: the *negated* diffs are computed
    d0' = x0-x1, d1' = x1-x2, d2' = x2-x3
(dots are bilinear, so g/t are unchanged) giving
    out = alpha*d1' + beta*d2' + d0',  [alpha, beta] = +K^-1 [g00, g01]
i.e. the final combine is two fused multiply-adds with positive signs.

Per-core plan (identical on all 8 cores — the whole job is five 8192-long
dot products, so replication beats sharding plus an AllReduce of 5 scalars):
    - the host interleaves x0..x3 to [128 partitions, 4 inputs, 64], so ONE
      fully-contiguous DMA (128 x 1KB descriptors, one HBM completion round
      trip) loads everything
    - one 3-lane tensor_sub for the negated diffs
    - products via two broadcast-view multiplies into [128,5,64], one
      tensor_reduce -> per-partition dots [128, 5]
    - ONE PE matmul with an all-ones [128,128] stationary both sums the
      partitions and broadcasts: every PSUM partition receives the column
      sums (duplicated twice via a stride-0 moving view -> [128,10]), so the
      2x2 solve runs replicated on all partitions and no second matmul or
      broadcast step exists
    - the 2x2 solve is 9 vector ops on [128,2]-shaped lanes (SIMD across
      partitions is free): one PSUM->SBUF copy, three pair-subtractions
      building [K11,-K01 | -K10,K00] and [K00,K10], two lane-product
      multiplies, one 3-lane reduce giving [a*detK, b*detK, detK], one
      reciprocal, one scale
    - two fused multiply-adds form the output, one contiguous store DMA
    - ops are ordered (and, in the serial solve tail, padded with anchored
      [1,1] spacer copies) so consumers never read the immediately preceding
      result: the DVE pipeline charges ~95ns forwarding delay at dependency
      distance 1, ~32ns at distance 2, nothing beyond
"""

import numpy as np

DIM = 8192
P = 128
F = DIM // P  # 64
N_CORES = 8

_cache = {}


def _build_module():
    from contextlib import ExitStack

    import concourse.bacc as bacc
    import concourse.tile as tile
    from concourse import mybir

    f32 = mybir.dt.float32
    nc = bacc.Bacc("TRN2", target_bir_lowering=False, debug=False)

    # Tile's kernel tail is drain -> all-engine barrier -> sem-clear ->
    # second all-engine barrier. NEFF completion already requires every
    # engine (including Pool, which runs the sem-clear) to reach the end of
    # its stream, and NRT serializes executions, so the second barrier only
    # adds ~200ns of tail latency. Skip it.
    _orig_dab = tile.TileContext._drain_and_barrier

    def _dab_one_barrier(self, tick_clock, wait_clock):
        calls = [0]
        orig_barrier = self.nc.all_engine_barrier

        def barrier_once(**kw):
            calls[0] += 1
            if calls[0] == 1:
                return orig_barrier(**kw)

        self.nc.all_engine_barrier = barrier_once
        try:
            return _orig_dab(self, tick_clock, wait_clock)
        finally:
            self.nc.all_engine_barrier = orig_barrier

    # Drop the framework's four const-AP memsets (const-float32-0.0 etc.):
    # nothing in this kernel reads them, and they sit on the Pool engine's
    # stream ahead of the init barrier, delaying the first DMA by ~370ns.
    bb0 = nc.m.functions[0].blocks[0]
    bb0.instructions[:] = [
        i for i in bb0.instructions if type(i).__name__ != "InstMemset"
    ]

    tile.TileContext._drain_and_barrier = _dab_one_barrier
    try:
        return _build_module_inner(nc, tile, mybir, f32, ExitStack)
    finally:
        tile.TileContext._drain_and_barrier = _orig_dab


def _build_module_inner(nc, tile, mybir, f32, ExitStack):
    xs = nc.dram_tensor("xs", [P, 4, F], f32, kind="ExternalInput").ap()
    out_dram = nc.dram_tensor("out", [DIM], f32, kind="ExternalOutput").ap()

    with tile.TileContext(nc) as tc, ExitStack() as ctx:
        _kernel_body(ctx, tc, out_dram, xs, f32, mybir)

    nc.compile()
    return nc


def _kernel_body(ctx, tc, out_dram, xs, f32, mybir):
    nc = tc.nc
    mult = mybir.AluOpType.mult
    add = mybir.AluOpType.add
    X = mybir.AxisListType.X

    sb = ctx.enter_context(tc.tile_pool(name="sb", bufs=1))
    ps = ctx.enter_context(tc.tile_pool(name="ps", bufs=1, space="PSUM"))

    # ---- load x0..x3 in ONE fully-contiguous DMA -------------------------
    # The host hands xs already interleaved as [partition, input, 64], so the
    # transfer is 128 descriptors of 1KB (vs 512x256B for per-input loads)
    # and pays a single HBM completion round trip.
    xt = sb.tile([P, 4, F], f32)
    nc.sync.dma_start(xt[:], xs[:])

    # negated diffs and five dots (prod lanes = d0'd0' d0'd1' d0'd2' | d1'd1'
    # d1'd2'). Ops are ordered so every consumer reads a result that is >= 2
    # instructions old: the DVE pipeline charges a ~95ns read-after-write
    # forwarding delay when an op consumes the immediately preceding output.
    cat = sb.tile([P, 3, F], f32)
    prod = sb.tile([P, 5, F], f32)
    r5 = sb.tile([P, 5], f32)  # cols: g00 g01 t01 g11 t11
    nc.vector.tensor_sub(cat[:, 1:3, :], xt[:, 1:3, :], xt[:, 2:4, :])
    nc.vector.tensor_sub(cat[:, 0, :], xt[:, 0, :], xt[:, 1, :])
    nc.vector.tensor_mul(
        prod[:, 3:5, :], cat[:, 1:2, :].broadcast_to([P, 2, F]), cat[:, 1:3, :]
    )
    nc.vector.tensor_mul(
        prod[:, 0:3, :], cat[:, 0:1, :].broadcast_to([P, 3, F]), cat[:, 0:3, :]
    )
    nc.vector.tensor_reduce(r5[:, 3:5], prod[:, 3:5, :], axis=X, op=add)
    nc.vector.tensor_reduce(r5[:, 0:3], prod[:, 0:3, :], axis=X, op=add)

    # Partition sums on the PE with an ALL-ONES stationary: every output
    # partition gets the same column sums, i.e. the matmul performs the
    # reduction AND the broadcast in one shot, so the 2x2 solve below runs
    # replicated on all 128 partitions and no second matmul is needed.
    # The moving operand is a stride-0-duplicated view of r5, so PSUM gets
    # [128,10] = the five sums twice on every partition.
    ones_sq = sb.tile([P, P], f32)
    nc.gpsimd.memset(ones_sq[:], 1.0)
    s2 = ps.tile([P, 10], f32)
    r5dup = r5[:].rearrange("p (o f) -> p o f", o=1).broadcast_to([P, 2, 5])
    nc.tensor.matmul(s2[:], ones_sq[:], r5dup)

    # ---- 2x2 solve: [alpha,beta] = K^-1 [g00,g01], replicated per partition
    # sc = the five sums twice, so any ordered scalar pair is a
    # positive-stride view. idx: 0=g00 1=g01 2=t01 3=g11 4=t11 (repeat 5..9)
    sc = sb.tile([P, 10], f32)
    nc.vector.tensor_copy(sc[:], s2[:])
    # Tiny spacer ops between each producer->consumer pair below: a consumer
    # of the immediately preceding result pays ~95ns of DVE pipeline
    # forwarding delay, a distance-2 consumer only ~32ns, so each 63ns spacer
    # nets about -32. Each spacer reads an output from >=2 ops earlier, which
    # pins its position in the engine stream without inheriting a hazard.
    spare = sb.tile([1, 8], f32)

    def _spacer(k, anchor):
        nc.vector.tensor_copy(spare[0:1, k : k + 1], anchor)

    _spacer(5, r5[0:1, 0:1])
    # kt = [K11, -K01 | -K10, K00]; ka = [K00, K10]
    kt = sb.tile([P, 4], f32)
    nc.vector.tensor_sub(kt[:, 0:2], sc[:, 3:8:4], sc[:, 4:7:2])  # [g11,t01]-[t11,g01]
    nc.vector.tensor_sub(kt[:, 2:4], sc[:, 3:6:2], sc[:, 1:7:5])  # [g11,g00]-[g01,g01]
    ka = sb.tile([P, 2], f32)
    nc.vector.tensor_sub(ka[:], sc[:, 0:2], sc[:, 1:4:2])  # [g00,g01]-[g01,g11]
    # pall = [kt * (g00,g01,g00,g01) | ka*ku]; one 3-lane reduce then gives
    # [alpha*detK, beta*detK, detK]
    pall = sb.tile([P, 6], f32)
    nc.vector.tensor_mul(
        pall[:, 0:4],
        kt[:],
        sc[:, 0:2].rearrange("p (o f) -> p o f", o=1).broadcast_to([P, 2, 2]),
    )
    nc.vector.tensor_mul(pall[:, 4:6], ka[:], kt[:, 0:2])
    _spacer(0, ka[0:1, 0:1])
    w3 = sb.tile([P, 3], f32)
    nc.vector.tensor_reduce(
        w3[:], pall[:].rearrange("p (a b) -> p a b", a=3), axis=X, op=add
    )
    _spacer(1, pall[0:1, 0:1])
    rdetk = sb.tile([P, 1], f32)
    nc.vector.reciprocal(rdetk[:], w3[:, 2:3])
    _spacer(2, w3[0:1, 0:1])
    ab_bc = sb.tile([P, 2], f32)
    nc.vector.tensor_scalar(ab_bc[:], w3[:, 0:2], rdetk[:, 0:1], None, mult)

    # out = alpha*d1' + beta*d2' + d0', then one contiguous store DMA
    _spacer(3, rdetk[0:1, 0:1])
    f1 = sb.tile([P, F], f32)
    nc.vector.scalar_tensor_tensor(
        f1[:], cat[:, 1, :], ab_bc[:, 0:1], cat[:, 0, :], mult, add
    )
    _spacer(4, ab_bc[0:1, 0:1])
    out_sb = sb.tile([P, F], f32)
    nc.vector.scalar_tensor_tensor(
        out_sb[:], cat[:, 2, :], ab_bc[:, 1:2], f1[:], mult, add
    )
    nc.sync.dma_start(out_dram.rearrange("(p f) -> p f", p=P), out_sb[:])


def _get_nc():
    if "nc" not in _cache:
        _cache["nc"] = _build_module()
    return _cache["nc"]


def run(in_map, trace=False):
    """Run the Bass kernel on all 8 NeuronCores (replicated inputs).

    The axon-tunneled device occasionally reports NRT_EXEC_UNIT_UNRECOVERABLE
    from stale terminal state and self-recovers within ~a minute, so a
    failed dispatch is retried (bounded) before giving up.
    """
    import time

    from concourse.bass_utils import run_bass_kernel_spmd

    nc = _get_nc()
    in_maps = [dict(in_map) for _ in range(N_CORES)]
    last_err = None
    for attempt in range(3):
        if attempt:
            time.sleep(45)
        try:
            return run_bass_kernel_spmd(
                nc, in_maps, core_ids=list(range(N_CORES)), trace=trace
            )
        except Exception as e:  # device wedge is transient; retry
            last_err = e
    raise last_err


def interleave(x0, x1, x2, x3):
    """[4, dim] host data -> [128, 4, 64] so the load is one contiguous DMA."""
    xs = np.stack(
        [np.asarray(v, dtype=np.float32).reshape(P, F) for v in (x0, x1, x2, x3)],
        axis=1,
    )
    return np.ascontiguousarray(xs)


def kernel(x0, x1, x2, x3):
    res = run({"xs": interleave(x0, x1, x2, x3)}, trace=False)
    return np.asarray(res.results[0]["out"], dtype=np.float32)
